# revision 7
# baseline (speedup 1.0000x reference)
"""Multi-head attention (b=4, n=2048, dim=1024, heads=16, hd=64) on 8 TRN2
NeuronCores.

Sharding: core i = (batch b = i//2, query-half h = i%2). Fully local — each
core recomputes K/V for its batch's full 2048 tokens (+25% FLOPs, zero
communication), computes Q for its 1024 tokens, per-head attention with
transposed scores S^T[k, q] (Q/K stay feature-major straight from the QKV
matmuls), softmax without max-subtraction (scores ~N(0, 0.33^2)), row sums via
an appended ones-column on V, then the out-projection. All matmul inputs bf16
(PSUM f32). The host does all sharding / transposes / bias folds in numpy and
reassembles the output.

Layouts on device (feature-major, partition dim first):
  xT   [128, 8 dc, 2048 t]   x^T, d-chunked; local-half tokens first
  qT   [128, 8 fc, 1024 t]   Q^T = wqT.T @ xT[:, :1024] + bq
  kT   [128, 8 fc, 2048 t]   K^T
  v    [128, 16 tt, 16 h, 65] V token-major per head, col 64 == 1.0 (sum row)
  S^T  psum [128 k, 512 q] = kT_h_slice.T @ qT_h_slice   (contraction hd=64)
  P~   exp(S^T/8) bf16; PV: psum_o[65, 512] += v_aug.T @ P~ (row 64 = sums)
  attn [128, 8 fc, 1024 t]   normalized, head-concat feature-major
  out  [1024 e, 1024 t]^T -> DMA'd as outT; host transposes back
"""
import sys

sys.path.insert(0, "/opt/trn_rl_repo")

import numpy as np
import ml_dtypes

import concourse.bass as bass
import concourse.tile as tile
from concourse import bacc, mybir
from concourse.bass_utils import run_bass_kernel_spmd

BF16 = mybir.dt.bfloat16
F32 = mybir.dt.float32
EXP = mybir.ActivationFunctionType.Exp
MULT = mybir.AluOpType.mult

D = 1024          # model dim
DC = 8            # d chunks of 128
NT = 2048         # kv tokens per core
NQ = 1024         # q tokens per core
NH = 16           # heads
HD = 64           # head dim
QC = 512          # q chunk (psum free)
NKT = 16          # k tiles of 128
N_CORES = 8

_CACHE = {}


def _install_ntff_shim():
    """The agent image's ``antenv`` lacks ``axon_hooks``, so concourse's
    trace=True path can't find the NTFF profile hook even though
    ``libaxon_pjrt.so`` supports it. Recreate the glue (same contract as
    trn_boot's ``_ntff_profile_via_ctypes``)."""
    import types
    import ctypes
    import contextlib

    if "antenv.axon_hooks" in sys.modules:
        return
    so_path = "/opt/axon/libaxon_pjrt.so"
    try:
        lib = ctypes.CDLL(so_path)
        if not hasattr(lib, "axon_start_nrt_profile"):
            return
    except OSError:
        return
    lib.axon_start_nrt_profile.argtypes = [ctypes.POINTER(ctypes.c_int64),
                                           ctypes.c_size_t]
    lib.axon_start_nrt_profile.restype = ctypes.c_int64
    lib.axon_stop_nrt_profile.argtypes = [ctypes.c_char_p]
    lib.axon_stop_nrt_profile.restype = ctypes.c_int64

    @contextlib.contextmanager
    def _hook(output_dir, device_ids):
        import jax
        jax.devices()
        if device_ids:
            ids = (ctypes.c_int64 * len(device_ids))(*device_ids)
            rc = lib.axon_start_nrt_profile(ids, len(device_ids))
        else:
            rc = lib.axon_start_nrt_profile(None, 0)
        if rc != 0:
            raise RuntimeError(f"axon_start_nrt_profile rc={rc}")
        try:
            yield
        finally:
            n = lib.axon_stop_nrt_profile(str(output_dir).encode())
            print(f"ntff profile: {n} file(s) written to {output_dir}",
                  file=sys.stderr)

    mod = types.ModuleType("antenv.axon_hooks")
    _h = [_hook]
    mod.set_axon_ntff_profile_hook = lambda h: _h.__setitem__(0, h)
    mod.get_axon_ntff_profile_hook = lambda: _h[0]
    sys.modules["antenv.axon_hooks"] = mod
    import antenv
    antenv.axon_hooks = mod


def build():
    nc = bacc.Bacc("TRN2", target_bir_lowering=False, debug=False,
                   num_devices=N_CORES)

    xT_d = nc.dram_tensor("xT", [D, NT], BF16, kind="ExternalInput")
    wq_d = nc.dram_tensor("wqT", [D, D], BF16, kind="ExternalInput")
    wk_d = nc.dram_tensor("wkT", [D, D], BF16, kind="ExternalInput")
    wv_d = nc.dram_tensor("wvT", [D, D], BF16, kind="ExternalInput")
    ow_d = nc.dram_tensor("owT", [D, D], BF16, kind="ExternalInput")
    bq_d = nc.dram_tensor("bq", [128, DC], F32, kind="ExternalInput")
    bk_d = nc.dram_tensor("bk", [128, DC], F32, kind="ExternalInput")
    ob_d = nc.dram_tensor("ob", [128, DC], F32, kind="ExternalInput")
    out_d = nc.dram_tensor("outT", [D, NQ], F32, kind="ExternalOutput")

    chunked = lambda t: t.ap().rearrange("(c p) t -> p c t", p=128)

    with tile.TileContext(nc) as tc:
        # ---------- persistent SBUF ----------
        with tc.tile_pool(name="persist", bufs=1) as persist:
            kT = persist.tile([128, DC, NT], BF16)
            qT = persist.tile([128, DC, NQ], BF16)
            v = persist.tile([128, NKT, NH, HD + 1], BF16)
            attn = persist.tile([128, DC, NQ], BF16)
            bq_sb = persist.tile([128, DC], F32)
            bk_sb = persist.tile([128, DC], F32)
            ob_sb = persist.tile([128, DC], F32)
            # ones columns for the PV sum row (v-proj epilogue writes skip col 64)
            nc.vector.memset(v, 1.0)
            warm = persist.tile([128, 1], F32)
            nc.vector.memset(warm, 0.0)

            # One static PSUM budget for the whole kernel (8 banks):
            #   ps_acc 2x[128,512] (proj + out-proj accumulators)    = 2
            #   ps_s   2x[128,2,512] (scores, even+odd head per kt)  = 4
            #   ps_o   2x[65,512]   (PV accumulators, even+odd head) = 2
            SB = 2  # heads per score batch (even/odd of a pair)
            with tc.tile_pool(name="w1", bufs=1) as w1, \
                 tc.tile_pool(name="xpool", bufs=1) as xpool, \
                 tc.tile_pool(name="ppool", bufs=3) as ppool, \
                 tc.tile_pool(name="nrm", bufs=2) as nrm, \
                 tc.tile_pool(name="fout", bufs=3) as fout, \
                 tc.tile_pool(name="drpool", bufs=4, space="DRAM") as drpool, \
                 tc.tile_pool(name="ps_acc", bufs=2, space="PSUM") as ps_acc, \
                 tc.tile_pool(name="ps_s", bufs=2, space="PSUM") as ps_s, \
                 tc.tile_pool(name="ps_o", bufs=2, space="PSUM") as ps_o:
                xT = xpool.tile([128, DC, NT], BF16)
                wq = w1.tile([128, DC, D], BF16, tag="wq")
                wk = w1.tile([128, DC, D], BF16, tag="wk")
                wv = w1.tile([128, DC, D], BF16, tag="wv")
                # ow shares wq's slot: loaded after Q-proj finishes with wq
                ow = w1.tile([128, DC, D], BF16, tag="wq", name="ow")
                # per-chunk loads so the first projection matmuls can start
                # as soon as chunk 0 lands; spread across three idle HWDGE
                # queues (the first K-proj chain needs all 8 chunks of
                # wk+xT, ~6MB, before its accumulation can finish)
                for dc in range(DC):
                    nc.scalar.dma_start(out=wk[:, dc, :],
                                        in_=chunked(wk_d)[:, dc, :])
                    nc.sync.dma_start(out=xT[:, dc, :],
                                      in_=chunked(xT_d)[:, dc, :])
                # biases after the first-matmul-critical chunk loads, before
                # the big weight transfers (first K epilogue needs bk ~25us)
                nc.sync.dma_start(out=bq_sb, in_=bq_d.ap())
                nc.sync.dma_start(out=bk_sb, in_=bk_d.ap())
                nc.sync.dma_start(out=ob_sb, in_=ob_d.ap())
                nc.sync.dma_start(out=wq, in_=chunked(wq_d))
                nc.sync.dma_start(out=wv, in_=chunked(wv_d))
                nc.sync.dma_start(out=ow, in_=chunked(ow_d))
                # dummy exp pulls the ~2.7us ACT_TABLE_LOAD off the first
                # real score tile's critical path; emitted AFTER the weight
                # DMA issues so it doesn't delay them on the ACT queue
                nc.scalar.activation(warm, warm, EXP)

                def proj_kq(fc):
                    # K^T / Q^T feature-chunk fc (feature-major)
                    for t4 in range(4):
                        ps = ps_acc.tile([128, QC], F32, tag="ps")
                        for dc in range(DC):
                            nc.tensor.matmul(
                                ps,
                                lhsT=wk[:, dc, fc * 128:(fc + 1) * 128],
                                rhs=xT[:, dc, t4 * QC:(t4 + 1) * QC],
                                start=(dc == 0), stop=(dc == DC - 1))
                        nc.vector.tensor_scalar_add(
                            kT[:, fc, t4 * QC:(t4 + 1) * QC], ps,
                            bk_sb[:, fc:fc + 1])
                    for t2 in range(2):
                        ps = ps_acc.tile([128, QC], F32, tag="ps")
                        for dc in range(DC):
                            nc.tensor.matmul(
                                ps,
                                lhsT=wq[:, dc, fc * 128:(fc + 1) * 128],
                                rhs=xT[:, dc, t2 * QC:(t2 + 1) * QC],
                                start=(dc == 0), stop=(dc == DC - 1))
                        nc.vector.tensor_scalar_add(
                            qT[:, fc, t2 * QC:(t2 + 1) * QC], ps,
                            bq_sb[:, fc:fc + 1])

                def proj_v(f2):
                    # V token-major, feature half f2 (heads 8*f2 .. 8*f2+7);
                    # epilogue into [.., 65]-strided per-head slots (ones col
                    # survives from the memset; bv folded into ob on host)
                    for tt in range(NKT):
                        ps = ps_acc.tile([128, QC], F32, tag="ps")
                        for dc in range(DC):
                            nc.tensor.matmul(
                                ps,
                                lhsT=xT[:, dc, tt * 128:(tt + 1) * 128],
                                rhs=wv[:, dc, f2 * QC:(f2 + 1) * QC],
                                start=(dc == 0), stop=(dc == DC - 1))
                        nc.vector.tensor_copy(
                            out=v[:, tt, f2 * 8:(f2 + 1) * 8, 0:HD],
                            in_=ps.rearrange("p (h d) -> p h d", d=HD))

                def attn_pair(fc, qc):
                    # Both heads of feature-chunk fc: even head at partitions
                    # 0:64, odd at 64:128.  Emitting the two score matmuls
                    # back-to-back puts them on disjoint PE row groups
                    # (tile_position (0,0) / (64,0)) so the HW runs them
                    # concurrently (~2x score throughput); their LDWEIGHTS
                    # pull ahead across row groups as well.
                    qsl = slice(qc * QC, (qc + 1) * QC)
                    po_e = ps_o.tile([HD + 1, QC], F32, tag="po")
                    po_o = ps_o.tile([HD + 1, QC], F32, tag="po")
                    def pv(pt, kt):
                        nc.tensor.matmul(
                            po_e,
                            lhsT=v[:, kt, 2 * fc, :],
                            rhs=pt[:, 0, :],
                            start=(kt == 0), stop=(kt == NKT - 1))
                        nc.tensor.matmul(
                            po_o,
                            lhsT=v[:, kt, 2 * fc + 1, :],
                            rhs=pt[:, 1, :],
                            start=(kt == 0), stop=(kt == NKT - 1))

                    # 1-deep software pipeline: scores(kt)+exp(kt) are emitted
                    # before pv(kt-1), so the PE's in-order queue never parks
                    # a PV matmul (gated on exp(kt-1)) ahead of the score
                    # matmuls that feed ACT's next exp — ACT stays saturated.
                    prev = None
                    for kt in range(NKT):
                        ss = ps_s.tile([128, SB, QC], F32, tag="ss")
                        for j in range(SB):
                            hi = j * 64
                            nc.tensor.matmul(
                                ss[:, j, :],
                                lhsT=kT[hi:hi + HD, fc,
                                        kt * 128:(kt + 1) * 128],
                                rhs=qT[hi:hi + HD, fc, qsl],
                                start=True, stop=True)
                        pt = ppool.tile([128, SB, QC], BF16, tag="pt")
                        nc.scalar.activation(pt, ss, EXP, scale=0.125)
                        if prev is not None:
                            pv(*prev)
                        prev = (pt, kt)
                    pv(*prev)
                    # Evacuate each PV accumulator to SBUF with one fast copy
                    # (frees the PSUM bank), then normalize from SBUF:
                    # 1/sum row, DRAM-bounce partition broadcast, multiply.
                    for hi, po in ((0, po_e), (64, po_o)):
                        ps_sb = nrm.tile([HD + 1, QC], F32, tag="ps_sb",
                                         bufs=3)
                        nc.vector.tensor_copy(out=ps_sb, in_=po)
                        rc = nrm.tile([128, QC], F32, tag="rc")
                        nc.vector.reciprocal(rc[HD:HD + 1, :],
                                             ps_sb[HD:HD + 1, :])
                        dr = drpool.tile([1, QC], F32, tag="dr")
                        nc.sync.dma_start(out=dr, in_=rc[HD:HD + 1, :])
                        bc = nrm.tile([64, QC], F32, tag="bc")
                        nc.sync.dma_start(
                            out=bc,
                            in_=bass.AP(tensor=dr.tensor, offset=dr.offset,
                                        ap=[[0, 64], dr.ap[1]]))
                        if hi == 0:
                            nc.vector.tensor_tensor(
                                out=attn[0:HD, fc, qsl],
                                in0=ps_sb[0:HD, :], in1=bc, op=MULT)
                        else:
                            sh = nrm.tile([64, QC], BF16, tag="sh")
                            nc.vector.tensor_tensor(
                                out=sh, in0=ps_sb[0:HD, :], in1=bc, op=MULT)
                            nc.sync.dma_start(out=attn[64:128, fc, qsl],
                                              in_=sh)

                def out_proj(ec, t2):
                    ps = ps_acc.tile([128, QC], F32, tag="ps")
                    for fc in range(DC):
                        nc.tensor.matmul(
                            ps,
                            lhsT=ow[:, fc, ec * 128:(ec + 1) * 128],
                            rhs=attn[:, fc, t2 * QC:(t2 + 1) * QC],
                            start=(fc == 0), stop=(fc == DC - 1))
                    fo = fout.tile([128, QC], F32, tag="fo")
                    # bias-add on ACT: it is idle after its last exp, exactly
                    # when DVE is the tail bottleneck
                    nc.scalar.activation(fo, ps,
                                         mybir.ActivationFunctionType.Identity,
                                         bias=ob_sb[:, ec:ec + 1])
                    nc.sync.dma_start(
                        out=out_d.ap()[ec * 128:(ec + 1) * 128,
                                       t2 * QC:(t2 + 1) * QC],
                        in_=fo)

                # Interleaved emission: attention for head pair (2fc-2, 2fc-1)
                # right after K/Q chunk fc lands, V halves as needed.
                # Interleaved emission as before, but the last two pairs run
                # qc=0 before any of their qc=1 work, so out-proj t2=0
                # becomes PE filler during the ACT-bound endgame (otherwise
                # the PE duty cycle collapses there, HAM halves the clock,
                # and the tail runs cold).
                proj_kq(0)
                proj_v(0)
                proj_kq(1)
                for fc in range(2, DC):
                    for qc in range(2):
                        attn_pair(fc - 2, qc)
                    if fc == 4:
                        proj_v(1)
                    proj_kq(fc)
                for qc in range(2):                 # pair 6, both qc
                    attn_pair(DC - 2, qc)
                attn_pair(DC - 1, 0)                # pair 7 qc0
                # pair 7 qc1 emitted BEFORE out-proj t2=0: out-proj's fc0-6
                # matmuls are dependency-ready anyway (all qc0 heads done)
                # and fill endgame PE gaps, but this queue order keeps the
                # last odd-head shift DMA ahead of out-proj's output DMAs
                # on the sync queue
                attn_pair(DC - 1, 1)
                for ec in range(DC):
                    out_proj(ec, 0)
                for ec in range(DC):
                    out_proj(ec, 1)

    nc.compile()
    return nc


def _prep_in_maps(x, qkv_w, qkv_b, out_w, out_b):
    bf = ml_dtypes.bfloat16
    wqT = np.ascontiguousarray(qkv_w[0:D].T).astype(bf)
    wkT = np.ascontiguousarray(qkv_w[D:2 * D].T).astype(bf)
    wvT = np.ascontiguousarray(qkv_w[2 * D:3 * D].T).astype(bf)
    owT = np.ascontiguousarray(out_w.T).astype(bf)
    bq = np.ascontiguousarray(qkv_b[0:D].reshape(DC, 128).T).astype(np.float32)
    bk = np.ascontiguousarray(qkv_b[D:2 * D].reshape(DC, 128).T).astype(np.float32)
    ob_eff = out_b + out_w @ qkv_b[2 * D:3 * D]
    ob = np.ascontiguousarray(ob_eff.reshape(DC, 128).T).astype(np.float32)

    in_maps = []
    for i in range(N_CORES):
        b, h = i // 2, i % 2
        xb = x[b]
        xp = np.concatenate([xb[h * NQ:(h + 1) * NQ],
                             xb[(1 - h) * NQ:(2 - h) * NQ]], 0)
        xT = np.ascontiguousarray(xp.T).astype(bf)
        in_maps.append(dict(xT=xT, wqT=wqT, wkT=wkT, wvT=wvT, owT=owT,
                            bq=bq, bk=bk, ob=ob))
    return in_maps


def run(x, qkv_w, qkv_b, out_w, out_b, trace=False):
    if trace:
        _install_ntff_shim()
    if "nc" not in _CACHE:
        _CACHE["nc"] = build()
    nc = _CACHE["nc"]
    in_maps = _prep_in_maps(np.asarray(x, np.float32),
                            np.asarray(qkv_w, np.float32),
                            np.asarray(qkv_b, np.float32),
                            np.asarray(out_w, np.float32),
                            np.asarray(out_b, np.float32))
    res = run_bass_kernel_spmd(nc, in_maps, core_ids=list(range(N_CORES)),
                               trace=trace)
    out = np.empty((4, 2048, D), np.float32)
    for i in range(N_CORES):
        b, h = i // 2, i % 2
        out[b, h * NQ:(h + 1) * NQ] = res.results[i]["outT"].T
    return out, res


def kernel(**inputs):
    out, _ = run(**inputs)
    return out



# revision 12
# speedup vs baseline: 1.4354x; 1.4354x over previous
"""Multi-head attention (b=4, n=2048, dim=1024, heads=16, hd=64) on 8 TRN2
NeuronCores.

Sharding: core i = (batch b = i//2, head-half hh = i%2). Each core computes
Q/K/V projections for its 8 heads only (column-split QKV — no duplicated
K/V work), full 2048x2048 attention for those heads, and a row-split
out-projection partial; the host sums the two partials per batch and adds
the (bv-folded) output bias.

Device layouts (feature-major, partition dim first):
  xT   [128, 8 dc, 2048 t]   x^T, d-chunked
  qT   [128, 4 fc, 2048 t]   Q^T local features (head pair p = chunk p)
  kT   [128, 4 fc, 2048 t]   K^T
  v    [128, 16 tt, 8 h, 65] V token-major per head, col 64 == 1.0 (sum row)
  S^T  psum [128 k, 2 h, 512 q] per k-tile: even head rows 0:64, odd 64:128
       of the PE array (tile_position row groups -> concurrent matmuls)
  P~   exp(S^T/8) bf16; PV: po[65, 512] += v_aug.T @ P~ (row 64 = sums)
  attn [128, 4 fc, 2048 t]   normalized, head-concat feature-major
  outT [1024 e, 2048 t] f32  partial (host sums core pairs, adds bias)

Schedule: a stream of 16 (pair, q-chunk) units x 16 k-tile slots. Each slot
emits the two row-tiled score matmuls + exp + previous slot's PV, plus
"filler" projection matmuls popped from a deadline-ordered queue so the PE
never idles long enough for the HAM clock gate to re-throttle. Unit 1 defers
its PV matmuls to its tail so the V-projection (its filler) can complete
under the exp stream instead of in a serial preamble.
"""
import sys

sys.path.insert(0, "/opt/trn_rl_repo")

from collections import deque

import numpy as np
import ml_dtypes

import concourse.bass as bass
import concourse.tile as tile
from concourse import bacc, mybir
from concourse.bass_utils import run_bass_kernel_spmd

BF16 = mybir.dt.bfloat16
F32 = mybir.dt.float32
EXP = mybir.ActivationFunctionType.Exp
MULT = mybir.AluOpType.mult

D = 1024          # model dim
DC = 8            # d chunks of 128
NT = 2048         # tokens per core (q and k)
FL = 512          # local features (8 heads)
FC = 4            # local feature chunks of 128
NH = 8            # local heads
NP = 4            # local head pairs
HD = 64           # head dim
QC = 512          # q chunk (psum free)
NQC = 4           # q chunks
NKT = 16          # k tiles of 128
SB = 2            # heads per score psum tile (even/odd)
N_CORES = 8

_CACHE = {}


def _install_ntff_shim():
    """The agent image's ``antenv`` lacks ``axon_hooks``, so concourse's
    trace=True path can't find the NTFF profile hook even though
    ``libaxon_pjrt.so`` supports it. Recreate the glue (same contract as
    trn_boot's ``_ntff_profile_via_ctypes``)."""
    import types
    import ctypes
    import contextlib

    if "antenv.axon_hooks" in sys.modules:
        return
    so_path = "/opt/axon/libaxon_pjrt.so"
    try:
        lib = ctypes.CDLL(so_path)
        if not hasattr(lib, "axon_start_nrt_profile"):
            return
    except OSError:
        return
    lib.axon_start_nrt_profile.argtypes = [ctypes.POINTER(ctypes.c_int64),
                                           ctypes.c_size_t]
    lib.axon_start_nrt_profile.restype = ctypes.c_int64
    lib.axon_stop_nrt_profile.argtypes = [ctypes.c_char_p]
    lib.axon_stop_nrt_profile.restype = ctypes.c_int64

    @contextlib.contextmanager
    def _hook(output_dir, device_ids):
        import jax
        jax.devices()
        if device_ids:
            ids = (ctypes.c_int64 * len(device_ids))(*device_ids)
            rc = lib.axon_start_nrt_profile(ids, len(device_ids))
        else:
            rc = lib.axon_start_nrt_profile(None, 0)
        if rc != 0:
            raise RuntimeError(f"axon_start_nrt_profile rc={rc}")
        try:
            yield
        finally:
            n = lib.axon_stop_nrt_profile(str(output_dir).encode())
            print(f"ntff profile: {n} file(s) written to {output_dir}",
                  file=sys.stderr)

    mod = types.ModuleType("antenv.axon_hooks")
    _h = [_hook]
    mod.set_axon_ntff_profile_hook = lambda h: _h.__setitem__(0, h)
    mod.get_axon_ntff_profile_hook = lambda: _h[0]
    sys.modules["antenv.axon_hooks"] = mod
    import antenv
    antenv.axon_hooks = mod


def build():
    nc = bacc.Bacc("TRN2", target_bir_lowering=False, debug=False,
                   num_devices=N_CORES)

    xT_d = nc.dram_tensor("xT", [D, NT], BF16, kind="ExternalInput")
    wq_d = nc.dram_tensor("wqT", [D, FL], BF16, kind="ExternalInput")
    wk_d = nc.dram_tensor("wkT", [D, FL], BF16, kind="ExternalInput")
    wv_d = nc.dram_tensor("wvT", [D, FL], BF16, kind="ExternalInput")
    ow_d = nc.dram_tensor("owT", [FL, D], BF16, kind="ExternalInput")
    bq_d = nc.dram_tensor("bq", [128, FC], F32, kind="ExternalInput")
    bk_d = nc.dram_tensor("bk", [128, FC], F32, kind="ExternalInput")
    out_d = nc.dram_tensor("outT", [D, NT], F32, kind="ExternalOutput")

    chunked = lambda t: t.ap().rearrange("(c p) t -> p c t", p=128)

    with tile.TileContext(nc) as tc:
        with tc.tile_pool(name="persist", bufs=1) as persist:
            kT = persist.tile([128, FC, NT], BF16)
            qT = persist.tile([128, FC, NT], BF16)
            v = persist.tile([128, NKT, NH, HD + 1], BF16)
            attn = persist.tile([128, FC, NT], BF16)
            bq_sb = persist.tile([128, FC], F32)
            bk_sb = persist.tile([128, FC], F32)
            nc.vector.memset(v, 1.0)
            warm = persist.tile([128, 1], F32)
            nc.vector.memset(warm, 0.0)

            # PSUM budget (8 banks): ps_acc 2x[128,512] proj/out accumulators,
            # ps_s 2x[128,2,512] scores, ps_o 2x[65,512] PV accumulators.
            with tc.tile_pool(name="w1", bufs=1) as w1, \
                 tc.tile_pool(name="xpool", bufs=1) as xpool, \
                 tc.tile_pool(name="ppool", bufs=17) as ppool, \
                 tc.tile_pool(name="nrm", bufs=2) as nrm, \
                 tc.tile_pool(name="fout", bufs=3) as fout, \
                 tc.tile_pool(name="drpool", bufs=4, space="DRAM") as drpool, \
                 tc.tile_pool(name="ps_acc", bufs=2, space="PSUM") as ps_acc, \
                 tc.tile_pool(name="ps_s", bufs=2, space="PSUM") as ps_s, \
                 tc.tile_pool(name="ps_o", bufs=2, space="PSUM") as ps_o:
                xT = xpool.tile([128, DC, NT], BF16)
                wq = w1.tile([128, DC, FL], BF16, tag="wq")
                wk = w1.tile([128, DC, FL], BF16, tag="wk")
                wv = w1.tile([128, DC, FL], BF16, tag="wv")
                ow = w1.tile([128, FC, D], BF16, tag="ow")

                # DMA: wk first (K proj gates the first scores), xT in
                # (dc, tc) pieces so early chains start before the full 4MB
                # lands. Weights on the ACT queue, xT on the sync queue.
                for dc in range(DC):
                    nc.scalar.dma_start(out=wk[:, dc, :],
                                        in_=chunked(wk_d)[:, dc, :])
                for tc_i in range(NQC):
                    for dc in range(DC):
                        nc.sync.dma_start(
                            out=xT[:, dc, tc_i * QC:(tc_i + 1) * QC],
                            in_=chunked(xT_d)[:, dc,
                                              tc_i * QC:(tc_i + 1) * QC])
                nc.scalar.dma_start(out=bq_sb, in_=bq_d.ap())
                nc.scalar.dma_start(out=bk_sb, in_=bk_d.ap())
                nc.scalar.dma_start(out=wq, in_=chunked(wq_d))
                nc.scalar.dma_start(out=wv, in_=chunked(wv_d))
                nc.scalar.dma_start(out=ow, in_=chunked(ow_d))
                # dummy exp pulls the ACT_TABLE_LOAD off the first real
                # score tile's critical path
                nc.scalar.activation(warm, warm, EXP)

                # ---- projection chains (8 matmuls + epilogue each) ----
                def k_chain(fc, tc_i):
                    tsl = slice(tc_i * QC, (tc_i + 1) * QC)
                    ps = ps_acc.tile([128, QC], F32, tag="ps")
                    for dc in range(DC):
                        yield nc.tensor.matmul(
                            ps, lhsT=wk[:, dc, fc * 128:(fc + 1) * 128],
                            rhs=xT[:, dc, tsl],
                            start=(dc == 0), stop=(dc == DC - 1))
                    yield nc.vector.tensor_scalar_add(
                        kT[:, fc, tsl], ps, bk_sb[:, fc:fc + 1])

                def q_chain(fc, tc_i):
                    tsl = slice(tc_i * QC, (tc_i + 1) * QC)
                    ps = ps_acc.tile([128, QC], F32, tag="ps")
                    for dc in range(DC):
                        yield nc.tensor.matmul(
                            ps, lhsT=wq[:, dc, fc * 128:(fc + 1) * 128],
                            rhs=xT[:, dc, tsl],
                            start=(dc == 0), stop=(dc == DC - 1))
                    yield nc.vector.tensor_scalar_add(
                        qT[:, fc, tsl], ps, bq_sb[:, fc:fc + 1])

                def v_chain(tt):
                    ps = ps_acc.tile([128, QC], F32, tag="ps")
                    for dc in range(DC):
                        yield nc.tensor.matmul(
                            ps, lhsT=xT[:, dc, tt * 128:(tt + 1) * 128],
                            rhs=wv[:, dc, :],
                            start=(dc == 0), stop=(dc == DC - 1))
                    yield nc.vector.tensor_copy(
                        out=v[:, tt, :, 0:HD],
                        in_=ps.rearrange("p (h d) -> p h d", d=HD))

                def out_chain(ec, tc_i):
                    tsl = slice(tc_i * QC, (tc_i + 1) * QC)
                    ps = ps_acc.tile([128, QC], F32, tag="ps")
                    for fc in range(FC):
                        yield nc.tensor.matmul(
                            ps, lhsT=ow[:, fc, ec * 128:(ec + 1) * 128],
                            rhs=attn[:, fc, tsl],
                            start=(fc == 0), stop=(fc == FC - 1))
                    fo = fout.tile([128, QC], F32, tag="fo")
                    yield nc.vector.tensor_copy(out=fo, in_=ps)
                    yield nc.sync.dma_start(
                        out=out_d.ap()[ec * 128:(ec + 1) * 128, tsl], in_=fo)

                # Deadline-ordered filler queue of (key, generator); attn
                # units pop a couple of steps per k-tile slot to keep the PE
                # dense while ACT owns the critical path.  Correctness rule:
                # everything a unit's own matmuls READ must be fully emitted
                # before the unit emits them (the PE executes in order, so a
                # score matmul parked on a not-yet-emitted chain's epilogue
                # deadlocks the queue) — require() force-drains those.
                filler = deque()
                done_keys = set()

                def push(key, gen):
                    filler.append((key, gen))

                def drain(n):
                    for _ in range(n):
                        if not filler:
                            return
                        key, gen = filler[0]
                        try:
                            next(gen)
                        except StopIteration:
                            done_keys.add(key)
                            filler.popleft()

                def drain_all():
                    while filler:
                        drain(1)

                def require(*keys):
                    while any(k not in done_keys for k in keys):
                        assert filler, f"missing filler chains: {keys}"
                        drain(1)

                def attn_unit(p, qc, defer_pv=False, fill=2):
                    require(*[("k", p, t) for t in range(NQC)],
                            ("q", p, qc))
                    if not defer_pv:
                        require(*[("v", tt) for tt in range(NKT)])
                    he, ho = 2 * p, 2 * p + 1
                    qsl = slice(qc * QC, (qc + 1) * QC)
                    po_e = ps_o.tile([HD + 1, QC], F32, tag="po")
                    po_o = ps_o.tile([HD + 1, QC], F32, tag="po")

                    def pv(pt, kt):
                        nc.tensor.matmul(
                            po_e, lhsT=v[:, kt, he, :], rhs=pt[:, 0, :],
                            start=(kt == 0), stop=(kt == NKT - 1))
                        nc.tensor.matmul(
                            po_o, lhsT=v[:, kt, ho, :], rhs=pt[:, 1, :],
                            start=(kt == 0), stop=(kt == NKT - 1))

                    backlog = []
                    prev = None
                    for kt in range(NKT):
                        ss = ps_s.tile([128, SB, QC], F32, tag="ss")
                        for j in range(SB):
                            hi = j * 64
                            nc.tensor.matmul(
                                ss[:, j, :],
                                lhsT=kT[hi:hi + HD, p,
                                        kt * 128:(kt + 1) * 128],
                                rhs=qT[hi:hi + HD, p, qsl],
                                start=True, stop=True)
                        pt = ppool.tile([128, SB, QC], BF16, tag="pt",
                                        bufs=17)
                        nc.scalar.activation(pt, ss, EXP, scale=0.125)
                        if defer_pv:
                            backlog.append((pt, kt))
                        else:
                            if prev is not None:
                                pv(*prev)
                            prev = (pt, kt)
                        drain(fill)
                    if defer_pv:
                        # the deferred PV reads v — every v chain must be
                        # emitted before these matmuls enter the PE queue
                        require(*[("v", tt) for tt in range(NKT)])
                        for b in backlog:
                            pv(*b)
                    else:
                        pv(*prev)

                    # normalization: evacuate both PV accumulators, batch the
                    # two 1/sum rows into one reciprocal, DRAM-bounce the
                    # partition broadcast, multiply.
                    ps_e = nrm.tile([HD + 1, QC], F32, tag="ps_sb", bufs=3)
                    nc.vector.tensor_copy(out=ps_e, in_=po_e)
                    ps_o_sb = nrm.tile([HD + 1, QC], F32, tag="ps_sb",
                                       bufs=3)
                    nc.vector.tensor_copy(out=ps_o_sb, in_=po_o)
                    # partition-gather the two sums rows via DMA (DVE ops
                    # cannot shift partition bases), one reciprocal for both
                    sr = nrm.tile([2, QC], F32, tag="sr")
                    nc.sync.dma_start(out=sr[0:1, :], in_=ps_e[HD:HD + 1, :])
                    nc.sync.dma_start(out=sr[1:2, :],
                                      in_=ps_o_sb[HD:HD + 1, :])
                    rc = nrm.tile([2, QC], F32, tag="rc")
                    nc.vector.reciprocal(rc, sr)
                    dr = drpool.tile([2, QC], F32, tag="dr")
                    nc.sync.dma_start(out=dr, in_=rc)
                    bc_e = nrm.tile([64, QC], F32, tag="bc_e")
                    nc.sync.dma_start(
                        out=bc_e,
                        in_=bass.AP(tensor=dr.tensor, offset=dr.offset,
                                    ap=[[0, 64], dr.ap[-1]]))
                    bc_o = nrm.tile([64, QC], F32, tag="bc_o")
                    nc.sync.dma_start(
                        out=bc_o,
                        in_=bass.AP(tensor=dr.tensor,
                                    offset=dr.offset + QC,
                                    ap=[[0, 64], dr.ap[-1]]))
                    nc.vector.tensor_tensor(
                        out=attn[0:HD, p, qsl],
                        in0=ps_e[0:HD, :], in1=bc_e, op=MULT)
                    sh = nrm.tile([64, QC], BF16, tag="sh")
                    nc.vector.tensor_tensor(
                        out=sh, in0=ps_o_sb[0:HD, :], in1=bc_o, op=MULT)
                    nc.sync.dma_start(out=attn[64:128, p, qsl], in_=sh)

                # ---- emission ----
                # preamble: K chunk 0 (all tokens) + Q chunk 0 for qc 0
                for tc_i in range(NQC):
                    push(("k", 0, tc_i), k_chain(0, tc_i))
                push(("q", 0, 0), q_chain(0, 0))
                require(*[("k", 0, t) for t in range(NQC)], ("q", 0, 0))

                # unit 1 runs with the V projection as its filler (PV
                # deferred to its tail); everything else streams normally.
                for tt in range(NKT):
                    push(("v", tt), v_chain(tt))
                push(("q", 0, 1), q_chain(0, 1))
                for tc_i in range(NQC):
                    push(("k", 1, tc_i), k_chain(1, tc_i))
                push(("q", 1, 0), q_chain(1, 0))
                attn_unit(0, 0, defer_pv=True, fill=9)

                push(("q", 1, 1), q_chain(1, 1))
                push(("q", 0, 2), q_chain(0, 2))
                attn_unit(0, 1)
                push(("q", 0, 3), q_chain(0, 3))
                push(("q", 1, 2), q_chain(1, 2))
                attn_unit(1, 0)
                for tc_i in range(NQC):
                    push(("k", 2, tc_i), k_chain(2, tc_i))
                attn_unit(1, 1)
                push(("q", 1, 3), q_chain(1, 3))
                push(("q", 2, 0), q_chain(2, 0))
                attn_unit(0, 2)
                push(("q", 2, 1), q_chain(2, 1))
                attn_unit(0, 3)
                for tc_i in range(NQC):
                    push(("k", 3, tc_i), k_chain(3, tc_i))
                attn_unit(1, 2)
                push(("q", 3, 0), q_chain(3, 0))
                push(("q", 3, 1), q_chain(3, 1))
                attn_unit(1, 3)
                push(("q", 2, 2), q_chain(2, 2))
                push(("q", 2, 3), q_chain(2, 3))
                attn_unit(2, 0)
                push(("q", 3, 2), q_chain(3, 2))
                push(("q", 3, 3), q_chain(3, 3))
                attn_unit(2, 1)
                attn_unit(3, 0)
                attn_unit(3, 1)
                # qc0/qc1 attn complete for all pairs -> out-proj tc 0, 1
                for ec in range(DC):
                    push(("o", ec, 0), out_chain(ec, 0))
                attn_unit(2, 2)
                for ec in range(DC):
                    push(("o", ec, 1), out_chain(ec, 1))
                attn_unit(2, 3)
                attn_unit(3, 2, fill=3)
                for ec in range(DC):
                    push(("o", ec, 2), out_chain(ec, 2))
                attn_unit(3, 3, fill=3)
                drain_all()
                for ec in range(DC):
                    push(("o", ec, 3), out_chain(ec, 3))
                drain_all()

    nc.compile()
    return nc


def _prep_in_maps(x, qkv_w, qkv_b, out_w, out_b):
    bf = ml_dtypes.bfloat16
    xTs = [np.ascontiguousarray(x[b].T).astype(bf) for b in range(4)]
    wqT, wkT, wvT, owT, bq, bk = [], [], [], [], [], []
    for hh in range(2):
        fsl = slice(hh * FL, (hh + 1) * FL)
        wqT.append(np.ascontiguousarray(qkv_w[0:D][fsl].T).astype(bf))
        wkT.append(np.ascontiguousarray(qkv_w[D:2 * D][fsl].T).astype(bf))
        wvT.append(np.ascontiguousarray(qkv_w[2 * D:3 * D][fsl].T).astype(bf))
        owT.append(np.ascontiguousarray(out_w.T[fsl]).astype(bf))
        bq.append(np.ascontiguousarray(
            qkv_b[0:D][fsl].reshape(FC, 128).T).astype(np.float32))
        bk.append(np.ascontiguousarray(
            qkv_b[D:2 * D][fsl].reshape(FC, 128).T).astype(np.float32))

    in_maps = []
    for i in range(N_CORES):
        b, hh = i // 2, i % 2
        in_maps.append(dict(xT=xTs[b], wqT=wqT[hh], wkT=wkT[hh],
                            wvT=wvT[hh], owT=owT[hh], bq=bq[hh], bk=bk[hh]))
    return in_maps


def run(x, qkv_w, qkv_b, out_w, out_b, trace=False):
    if trace:
        _install_ntff_shim()
    if "nc" not in _CACHE:
        _CACHE["nc"] = build()
    nc = _CACHE["nc"]
    x = np.asarray(x, np.float32)
    qkv_w = np.asarray(qkv_w, np.float32)
    qkv_b = np.asarray(qkv_b, np.float32)
    out_w = np.asarray(out_w, np.float32)
    out_b = np.asarray(out_b, np.float32)
    in_maps = _prep_in_maps(x, qkv_w, qkv_b, out_w, out_b)
    res = run_bass_kernel_spmd(nc, in_maps, core_ids=list(range(N_CORES)),
                               trace=trace)
    # host: sum the two head-half partials per batch, add bv-folded bias
    ob_eff = (out_b + out_w @ qkv_b[2 * D:3 * D]).astype(np.float32)
    out = np.empty((4, NT, D), np.float32)
    for b in range(4):
        acc = res.results[2 * b]["outT"] + res.results[2 * b + 1]["outT"]
        out[b] = acc.T + ob_eff
    return out, res


def kernel(**inputs):
    out, _ = run(**inputs)
    return out


# revision 17
# speedup vs baseline: 1.4698x; 1.0239x over previous
"""Multi-head attention (b=4, n=2048, dim=1024, heads=16, hd=64) on 8 TRN2
NeuronCores.

Sharding: core i = (batch b = i//2, head-half hh = i%2). Each core computes
Q/K/V projections for its 8 heads only (column-split QKV — no duplicated
K/V work), full 2048x2048 attention for those heads, and a row-split
out-projection partial; the host sums the two partials per batch and adds
the (bv-folded) output bias.

Device layouts (feature-major, partition dim first):
  xT   [128, 8 dc, 2048 t]   x^T, d-chunked
  qT   [128, 4 fc, 2048 t]   Q^T local features (head pair p = chunk p)
  kT   [128, 4 fc, 2048 t]   K^T
  v    [128, 16 tt, 8 h, 65] V token-major per head, col 64 == 1.0 (sum row)
  S^T  psum [128 k, 2 h, 512 q] per k-tile: even head rows 0:64, odd 64:128
       of the PE array (tile_position row groups -> concurrent matmuls)
  P~   exp(S^T/8) bf16; PV: po[65, 512] += v_aug.T @ P~ (row 64 = sums)
  attn [128, 4 fc, 2048 t]   normalized, head-concat feature-major
  outT [1024 e, 2048 t] f32  partial (host sums core pairs, adds bias)

Schedule: a stream of 16 (pair, q-chunk) units x 16 k-tile slots. Each slot
emits the two row-tiled score matmuls + exp + previous slot's PV, plus
"filler" projection matmuls popped from a deadline-ordered queue so the PE
never idles long enough for the HAM clock gate to re-throttle. Unit 1 defers
its PV matmuls to its tail so the V-projection (its filler) can complete
under the exp stream instead of in a serial preamble.
"""
import sys

sys.path.insert(0, "/opt/trn_rl_repo")

from collections import deque

import numpy as np
import ml_dtypes

import concourse.bass as bass
import concourse.tile as tile
from concourse import bacc, mybir
from concourse.bass_utils import run_bass_kernel_spmd

BF16 = mybir.dt.bfloat16
F32 = mybir.dt.float32
EXP = mybir.ActivationFunctionType.Exp
MULT = mybir.AluOpType.mult

D = 1024          # model dim
DC = 8            # d chunks of 128
NT = 2048         # tokens per core (q and k)
FL = 512          # local features (8 heads)
FC = 4            # local feature chunks of 128
NH = 8            # local heads
NP = 4            # local head pairs
HD = 64           # head dim
QC = 512          # q chunk (psum free)
NQC = 4           # q chunks
NKT = 16          # k tiles of 128
SB = 2            # heads per score psum tile (even/odd)
N_CORES = 8

_CACHE = {}


def _install_ntff_shim():
    """The agent image's ``antenv`` lacks ``axon_hooks``, so concourse's
    trace=True path can't find the NTFF profile hook even though
    ``libaxon_pjrt.so`` supports it. Recreate the glue (same contract as
    trn_boot's ``_ntff_profile_via_ctypes``)."""
    import types
    import ctypes
    import contextlib

    if "antenv.axon_hooks" in sys.modules:
        return
    so_path = "/opt/axon/libaxon_pjrt.so"
    try:
        lib = ctypes.CDLL(so_path)
        if not hasattr(lib, "axon_start_nrt_profile"):
            return
    except OSError:
        return
    lib.axon_start_nrt_profile.argtypes = [ctypes.POINTER(ctypes.c_int64),
                                           ctypes.c_size_t]
    lib.axon_start_nrt_profile.restype = ctypes.c_int64
    lib.axon_stop_nrt_profile.argtypes = [ctypes.c_char_p]
    lib.axon_stop_nrt_profile.restype = ctypes.c_int64

    @contextlib.contextmanager
    def _hook(output_dir, device_ids):
        import jax
        jax.devices()
        if device_ids:
            ids = (ctypes.c_int64 * len(device_ids))(*device_ids)
            rc = lib.axon_start_nrt_profile(ids, len(device_ids))
        else:
            rc = lib.axon_start_nrt_profile(None, 0)
        if rc != 0:
            raise RuntimeError(f"axon_start_nrt_profile rc={rc}")
        try:
            yield
        finally:
            n = lib.axon_stop_nrt_profile(str(output_dir).encode())
            print(f"ntff profile: {n} file(s) written to {output_dir}",
                  file=sys.stderr)

    mod = types.ModuleType("antenv.axon_hooks")
    _h = [_hook]
    mod.set_axon_ntff_profile_hook = lambda h: _h.__setitem__(0, h)
    mod.get_axon_ntff_profile_hook = lambda: _h[0]
    sys.modules["antenv.axon_hooks"] = mod
    import antenv
    antenv.axon_hooks = mod


def build():
    nc = bacc.Bacc("TRN2", target_bir_lowering=False, debug=False,
                   num_devices=N_CORES)

    xT_d = nc.dram_tensor("xT", [D, NT], BF16, kind="ExternalInput")
    wq_d = nc.dram_tensor("wqT", [D, FL], BF16, kind="ExternalInput")
    wk_d = nc.dram_tensor("wkT", [D, FL], BF16, kind="ExternalInput")
    wv_d = nc.dram_tensor("wvT", [D, FL], BF16, kind="ExternalInput")
    ow_d = nc.dram_tensor("owT", [FL, D], BF16, kind="ExternalInput")
    bq_d = nc.dram_tensor("bq", [128, FC], F32, kind="ExternalInput")
    bk_d = nc.dram_tensor("bk", [128, FC], F32, kind="ExternalInput")
    out_d = nc.dram_tensor("outT", [D, NT], F32, kind="ExternalOutput")

    chunked = lambda t: t.ap().rearrange("(c p) t -> p c t", p=128)

    with tile.TileContext(nc) as tc:
        with tc.tile_pool(name="persist", bufs=1) as persist:
            kT = persist.tile([128, FC, NT], BF16)
            qT = persist.tile([128, FC, NT], BF16)
            v = persist.tile([128, NKT, NH, HD + 1], BF16)
            attn = persist.tile([128, FC, NT], BF16)
            bq_sb = persist.tile([128, FC], F32)
            bk_sb = persist.tile([128, FC], F32)
            nc.vector.memset(v, 1.0)
            warm = persist.tile([128, 1], F32)
            nc.vector.memset(warm, 0.0)

            # PSUM budget (8 banks): ps_acc 2x[128,512] proj/out accumulators,
            # ps_s 2x[128,2,512] scores, ps_o 2x[65,512] PV accumulators.
            with tc.tile_pool(name="w1", bufs=1) as w1, \
                 tc.tile_pool(name="xpool", bufs=1) as xpool, \
                 tc.tile_pool(name="ppool", bufs=17) as ppool, \
                 tc.tile_pool(name="nrm", bufs=2) as nrm, \
                 tc.tile_pool(name="fout", bufs=3) as fout, \
                 tc.tile_pool(name="drpool", bufs=4, space="DRAM") as drpool, \
                 tc.tile_pool(name="ps_acc", bufs=2, space="PSUM") as ps_acc, \
                 tc.tile_pool(name="ps_s", bufs=2, space="PSUM") as ps_s, \
                 tc.tile_pool(name="ps_o", bufs=2, space="PSUM") as ps_o:
                xT = xpool.tile([128, DC, NT], BF16)
                wq = w1.tile([128, DC, FL], BF16, tag="wq")
                wk = w1.tile([128, DC, FL], BF16, tag="wk")
                wv = w1.tile([128, DC, FL], BF16, tag="wv")
                ow = w1.tile([128, FC, D], BF16, tag="ow")

                # DMA: few, large transfers — the SP engine pays ~565ns of
                # issue time per dma_start, so 40 small loads would gate the
                # preamble on the issue rate alone. Weights on the ACT
                # queue, xT per-tc on the sync queue (tc0 lands first so the
                # first K chain starts ~4us in).
                nc.scalar.dma_start(out=bq_sb, in_=bq_d.ap())
                nc.scalar.dma_start(out=bk_sb, in_=bk_d.ap())
                nc.scalar.dma_start(out=wk, in_=chunked(wk_d))
                for tc_i in range(NQC):
                    nc.sync.dma_start(
                        out=xT[:, :, tc_i * QC:(tc_i + 1) * QC],
                        in_=chunked(xT_d)[:, :, tc_i * QC:(tc_i + 1) * QC])
                nc.scalar.dma_start(out=wq, in_=chunked(wq_d))
                nc.scalar.dma_start(out=wv, in_=chunked(wv_d))
                nc.scalar.dma_start(out=ow, in_=chunked(ow_d))
                # dummy exp pulls the ACT_TABLE_LOAD off the first real
                # score tile's critical path
                nc.scalar.activation(warm, warm, EXP)

                # ---- projection chains (8 matmuls + epilogue each) ----
                def k_chain(fc, tc_i):
                    tsl = slice(tc_i * QC, (tc_i + 1) * QC)
                    ps = ps_acc.tile([128, QC], F32, tag="ps")
                    for dc in range(DC):
                        yield nc.tensor.matmul(
                            ps, lhsT=wk[:, dc, fc * 128:(fc + 1) * 128],
                            rhs=xT[:, dc, tsl],
                            start=(dc == 0), stop=(dc == DC - 1))
                    yield nc.vector.tensor_scalar_add(
                        kT[:, fc, tsl], ps, bk_sb[:, fc:fc + 1])

                def q_chain(fc, tc_i):
                    tsl = slice(tc_i * QC, (tc_i + 1) * QC)
                    ps = ps_acc.tile([128, QC], F32, tag="ps")
                    for dc in range(DC):
                        yield nc.tensor.matmul(
                            ps, lhsT=wq[:, dc, fc * 128:(fc + 1) * 128],
                            rhs=xT[:, dc, tsl],
                            start=(dc == 0), stop=(dc == DC - 1))
                    yield nc.vector.tensor_scalar_add(
                        qT[:, fc, tsl], ps, bq_sb[:, fc:fc + 1])

                def v_chain(tt):
                    ps = ps_acc.tile([128, QC], F32, tag="ps")
                    for dc in range(DC):
                        yield nc.tensor.matmul(
                            ps, lhsT=xT[:, dc, tt * 128:(tt + 1) * 128],
                            rhs=wv[:, dc, :],
                            start=(dc == 0), stop=(dc == DC - 1))
                    yield nc.vector.tensor_copy(
                        out=v[:, tt, :, 0:HD],
                        in_=ps.rearrange("p (h d) -> p h d", d=HD))

                def out_chain(ec, tc_i):
                    tsl = slice(tc_i * QC, (tc_i + 1) * QC)
                    ps = ps_acc.tile([128, QC], F32, tag="ps")
                    for fc in range(FC):
                        yield nc.tensor.matmul(
                            ps, lhsT=ow[:, fc, ec * 128:(ec + 1) * 128],
                            rhs=attn[:, fc, tsl],
                            start=(fc == 0), stop=(fc == FC - 1))
                    fo = fout.tile([128, QC], F32, tag="fo")
                    yield nc.vector.tensor_copy(out=fo, in_=ps)
                    yield nc.sync.dma_start(
                        out=out_d.ap()[ec * 128:(ec + 1) * 128, tsl], in_=fo)

                # Deadline-ordered filler queue of (key, generator); attn
                # units pop a couple of steps per k-tile slot to keep the PE
                # dense while ACT owns the critical path.  Correctness rule:
                # everything a unit's own matmuls READ must be fully emitted
                # before the unit emits them (the PE executes in order, so a
                # score matmul parked on a not-yet-emitted chain's epilogue
                # deadlocks the queue) — require() force-drains those.
                filler = deque()
                done_keys = set()

                def push(key, gen):
                    filler.append((key, gen))

                def drain(n):
                    for _ in range(n):
                        if not filler:
                            return
                        key, gen = filler[0]
                        try:
                            next(gen)
                        except StopIteration:
                            done_keys.add(key)
                            filler.popleft()

                def drain_all():
                    while filler:
                        drain(1)

                def require(*keys):
                    while any(k not in done_keys for k in keys):
                        assert filler, f"missing filler chains: {keys}"
                        drain(1)

                def attn_unit(p, qc, defer_pv=False, fill=2):
                    # K chunks are required in kt-stages (kt//4 == tc) so the
                    # first scores don't wait on the whole 2048-token K
                    require(("k", p, 0), ("q", p, qc))
                    if not defer_pv:
                        require(*[("v", tt) for tt in range(NKT)])
                    he, ho = 2 * p, 2 * p + 1
                    qsl = slice(qc * QC, (qc + 1) * QC)
                    po_e = ps_o.tile([HD + 1, QC], F32, tag="po")
                    po_o = ps_o.tile([HD + 1, QC], F32, tag="po")

                    def pv(pt, kt):
                        nc.tensor.matmul(
                            po_e, lhsT=v[:, kt, he, :], rhs=pt[:, 0, :],
                            start=(kt == 0), stop=(kt == NKT - 1))
                        nc.tensor.matmul(
                            po_o, lhsT=v[:, kt, ho, :], rhs=pt[:, 1, :],
                            start=(kt == 0), stop=(kt == NKT - 1))

                    backlog = []
                    prev = None
                    for kt in range(NKT):
                        if kt % 4 == 0 and kt > 0:
                            require(("k", p, kt // 4))
                        ss = ps_s.tile([128, SB, QC], F32, tag="ss")
                        for j in range(SB):
                            hi = j * 64
                            nc.tensor.matmul(
                                ss[:, j, :],
                                lhsT=kT[hi:hi + HD, p,
                                        kt * 128:(kt + 1) * 128],
                                rhs=qT[hi:hi + HD, p, qsl],
                                start=True, stop=True)
                        pt = ppool.tile([128, SB, QC], BF16, tag="pt",
                                        bufs=17)
                        nc.scalar.activation(pt, ss, EXP, scale=0.125)
                        if defer_pv:
                            backlog.append((pt, kt))
                        else:
                            if prev is not None:
                                pv(*prev)
                            prev = (pt, kt)
                        drain(fill)
                    if defer_pv:
                        # the deferred PV reads v — every v chain must be
                        # emitted before these matmuls enter the PE queue
                        require(*[("v", tt) for tt in range(NKT)])
                        for b in backlog:
                            pv(*b)
                    else:
                        pv(*prev)

                    # normalization: evacuate both PV accumulators, batch the
                    # two 1/sum rows into one reciprocal, DRAM-bounce the
                    # partition broadcast, multiply.
                    ps_e = nrm.tile([HD + 1, QC], F32, tag="ps_sb", bufs=3)
                    nc.vector.tensor_copy(out=ps_e, in_=po_e)
                    ps_o_sb = nrm.tile([HD + 1, QC], F32, tag="ps_sb",
                                       bufs=3)
                    nc.vector.tensor_copy(out=ps_o_sb, in_=po_o)
                    # partition-gather the two sums rows via DMA (DVE ops
                    # cannot shift partition bases), one reciprocal for both
                    sr = nrm.tile([2, QC], F32, tag="sr")
                    nc.sync.dma_start(out=sr[0:1, :], in_=ps_e[HD:HD + 1, :])
                    nc.sync.dma_start(out=sr[1:2, :],
                                      in_=ps_o_sb[HD:HD + 1, :])
                    rc = nrm.tile([2, QC], F32, tag="rc")
                    nc.vector.reciprocal(rc, sr)
                    dr = drpool.tile([2, QC], F32, tag="dr")
                    nc.sync.dma_start(out=dr, in_=rc)
                    bc_e = nrm.tile([64, QC], F32, tag="bc_e")
                    nc.sync.dma_start(
                        out=bc_e,
                        in_=bass.AP(tensor=dr.tensor, offset=dr.offset,
                                    ap=[[0, 64], dr.ap[-1]]))
                    bc_o = nrm.tile([64, QC], F32, tag="bc_o")
                    nc.sync.dma_start(
                        out=bc_o,
                        in_=bass.AP(tensor=dr.tensor,
                                    offset=dr.offset + QC,
                                    ap=[[0, 64], dr.ap[-1]]))
                    nc.vector.tensor_tensor(
                        out=attn[0:HD, p, qsl],
                        in0=ps_e[0:HD, :], in1=bc_e, op=MULT)
                    sh = nrm.tile([64, QC], BF16, tag="sh")
                    nc.vector.tensor_tensor(
                        out=sh, in0=ps_o_sb[0:HD, :], in1=bc_o, op=MULT)
                    nc.sync.dma_start(out=attn[64:128, p, qsl], in_=sh)

                # ---- emission ----
                # preamble: only K(0, tc0) + Q(0, qc0) gate the first scores
                push(("k", 0, 0), k_chain(0, 0))
                push(("q", 0, 0), q_chain(0, 0))
                require(("k", 0, 0), ("q", 0, 0))

                # unit 1 runs with the remaining K chunks + the V projection
                # as its filler (PV deferred to its tail)
                for tc_i in range(1, NQC):
                    push(("k", 0, tc_i), k_chain(0, tc_i))
                for tt in range(NKT):
                    push(("v", tt), v_chain(tt))
                push(("q", 0, 1), q_chain(0, 1))
                for tc_i in range(NQC):
                    push(("k", 1, tc_i), k_chain(1, tc_i))
                push(("q", 1, 0), q_chain(1, 0))
                attn_unit(0, 0, defer_pv=True, fill=9)

                push(("q", 1, 1), q_chain(1, 1))
                push(("q", 0, 2), q_chain(0, 2))
                attn_unit(0, 1)
                push(("q", 0, 3), q_chain(0, 3))
                push(("q", 1, 2), q_chain(1, 2))
                attn_unit(1, 0)
                for tc_i in range(NQC):
                    push(("k", 2, tc_i), k_chain(2, tc_i))
                attn_unit(1, 1)
                push(("q", 1, 3), q_chain(1, 3))
                push(("q", 2, 0), q_chain(2, 0))
                attn_unit(0, 2)
                push(("q", 2, 1), q_chain(2, 1))
                attn_unit(0, 3)
                for tc_i in range(NQC):
                    push(("k", 3, tc_i), k_chain(3, tc_i))
                attn_unit(1, 2)
                push(("q", 3, 0), q_chain(3, 0))
                push(("q", 3, 1), q_chain(3, 1))
                attn_unit(1, 3)
                push(("q", 2, 2), q_chain(2, 2))
                push(("q", 2, 3), q_chain(2, 3))
                attn_unit(2, 0)
                push(("q", 3, 2), q_chain(3, 2))
                push(("q", 3, 3), q_chain(3, 3))
                attn_unit(2, 1)
                attn_unit(3, 0)
                attn_unit(3, 1)
                # qc0/qc1 attn complete for all pairs -> out-proj tc 0, 1
                for ec in range(DC):
                    push(("o", ec, 0), out_chain(ec, 0))
                attn_unit(2, 2)
                for ec in range(DC):
                    push(("o", ec, 1), out_chain(ec, 1))
                attn_unit(2, 3)
                attn_unit(3, 2, fill=3)
                # only 2 of the tc2 out-chains ride the last unit's slots;
                # the other 6 execute during the final normalization's
                # ~8us latency window, keeping the HAM clock gate warm so
                # the tc3 tail runs at full clock
                for ec in range(2):
                    push(("o", ec, 2), out_chain(ec, 2))
                attn_unit(3, 3, fill=2)
                for ec in range(2, DC):
                    push(("o", ec, 2), out_chain(ec, 2))
                drain_all()
                for ec in range(DC):
                    push(("o", ec, 3), out_chain(ec, 3))
                drain_all()

    nc.compile()
    return nc


def _prep_in_maps(x, qkv_w, qkv_b, out_w, out_b):
    bf = ml_dtypes.bfloat16
    xTs = [np.ascontiguousarray(x[b].T).astype(bf) for b in range(4)]
    wqT, wkT, wvT, owT, bq, bk = [], [], [], [], [], []
    for hh in range(2):
        fsl = slice(hh * FL, (hh + 1) * FL)
        wqT.append(np.ascontiguousarray(qkv_w[0:D][fsl].T).astype(bf))
        wkT.append(np.ascontiguousarray(qkv_w[D:2 * D][fsl].T).astype(bf))
        wvT.append(np.ascontiguousarray(qkv_w[2 * D:3 * D][fsl].T).astype(bf))
        owT.append(np.ascontiguousarray(out_w.T[fsl]).astype(bf))
        bq.append(np.ascontiguousarray(
            qkv_b[0:D][fsl].reshape(FC, 128).T).astype(np.float32))
        bk.append(np.ascontiguousarray(
            qkv_b[D:2 * D][fsl].reshape(FC, 128).T).astype(np.float32))

    in_maps = []
    for i in range(N_CORES):
        b, hh = i // 2, i % 2
        in_maps.append(dict(xT=xTs[b], wqT=wqT[hh], wkT=wkT[hh],
                            wvT=wvT[hh], owT=owT[hh], bq=bq[hh], bk=bk[hh]))
    return in_maps


def run(x, qkv_w, qkv_b, out_w, out_b, trace=False):
    if trace:
        _install_ntff_shim()
    if "nc" not in _CACHE:
        _CACHE["nc"] = build()
    nc = _CACHE["nc"]
    x = np.asarray(x, np.float32)
    qkv_w = np.asarray(qkv_w, np.float32)
    qkv_b = np.asarray(qkv_b, np.float32)
    out_w = np.asarray(out_w, np.float32)
    out_b = np.asarray(out_b, np.float32)
    in_maps = _prep_in_maps(x, qkv_w, qkv_b, out_w, out_b)
    res = run_bass_kernel_spmd(nc, in_maps, core_ids=list(range(N_CORES)),
                               trace=trace)
    # host: sum the two head-half partials per batch, add bv-folded bias
    ob_eff = (out_b + out_w @ qkv_b[2 * D:3 * D]).astype(np.float32)
    out = np.empty((4, NT, D), np.float32)
    for b in range(4):
        acc = res.results[2 * b]["outT"] + res.results[2 * b + 1]["outT"]
        out[b] = acc.T + ob_eff
    return out, res


def kernel(**inputs):
    out, _ = run(**inputs)
    return out


# revision 28
# speedup vs baseline: 1.5136x; 1.0298x over previous
"""Multi-head attention (b=4, n=2048, dim=1024, heads=16, hd=64) on 8 TRN2
NeuronCores.

Sharding: core i = (batch b = i//2, head-half hh = i%2). Each core computes
Q/K/V projections for its 8 heads only (column-split QKV — no duplicated
K/V work), full 2048x2048 attention for those heads, and a row-split
out-projection partial; the host sums the two partials per batch and adds
the (bv-folded) output bias.

Device layouts (feature-major, partition dim first):
  xT   [128, 8 dc, 2048 t]   x^T, d-chunked
  qT   [128, 4 fc, 2048 t]   Q^T local features (head pair p = chunk p)
  kT   [128, 4 fc, 2048 t]   K^T
  v    [128, 16 tt, 8 h, 65] V token-major per head, col 64 == 1.0 (sum row)
  S^T  psum [128 k, 2 h, 512 q] per k-tile: even head rows 0:64, odd 64:128
       of the PE array (tile_position row groups -> concurrent matmuls)
  P~   exp(S^T/8) bf16; PV: po[65, 512] += v_aug.T @ P~ (row 64 = sums)
  attn [128, 4 fc, 2048 t]   normalized, head-concat feature-major
  outT [1024 e, 2048 t] f32  partial (host sums core pairs, adds bias)

Schedule: a stream of 16 (pair, q-chunk) units x 16 k-tile slots. Each slot
emits the two row-tiled score matmuls + exp + previous slot's PV, plus
"filler" projection matmuls popped from a deadline-ordered queue so the PE
never idles long enough for the HAM clock gate to re-throttle. Unit 1 defers
its PV matmuls to its tail so the V-projection (its filler) can complete
under the exp stream instead of in a serial preamble.
"""
import sys

sys.path.insert(0, "/opt/trn_rl_repo")

from collections import deque

import numpy as np
import ml_dtypes

import concourse.bass as bass
import concourse.tile as tile
from concourse import bacc, mybir
from concourse.bass_utils import run_bass_kernel_spmd

BF16 = mybir.dt.bfloat16
F32 = mybir.dt.float32
EXP = mybir.ActivationFunctionType.Exp
MULT = mybir.AluOpType.mult

D = 1024          # model dim
DC = 8            # d chunks of 128
NT = 2048         # tokens per core (q and k)
FL = 512          # local features (8 heads)
FC = 4            # local feature chunks of 128
NH = 8            # local heads
NP = 4            # local head pairs
HD = 64           # head dim
QC = 512          # q chunk (psum free)
NQC = 4           # q chunks
NKT = 16          # k tiles of 128
SB = 2            # heads per score psum tile (even/odd)
N_CORES = 8

_CACHE = {}


def _install_ntff_shim():
    """The agent image's ``antenv`` lacks ``axon_hooks``, so concourse's
    trace=True path can't find the NTFF profile hook even though
    ``libaxon_pjrt.so`` supports it. Recreate the glue (same contract as
    trn_boot's ``_ntff_profile_via_ctypes``)."""
    import types
    import ctypes
    import contextlib

    if "antenv.axon_hooks" in sys.modules:
        return
    so_path = "/opt/axon/libaxon_pjrt.so"
    try:
        lib = ctypes.CDLL(so_path)
        if not hasattr(lib, "axon_start_nrt_profile"):
            return
    except OSError:
        return
    lib.axon_start_nrt_profile.argtypes = [ctypes.POINTER(ctypes.c_int64),
                                           ctypes.c_size_t]
    lib.axon_start_nrt_profile.restype = ctypes.c_int64
    lib.axon_stop_nrt_profile.argtypes = [ctypes.c_char_p]
    lib.axon_stop_nrt_profile.restype = ctypes.c_int64

    @contextlib.contextmanager
    def _hook(output_dir, device_ids):
        import jax
        jax.devices()
        if device_ids:
            ids = (ctypes.c_int64 * len(device_ids))(*device_ids)
            rc = lib.axon_start_nrt_profile(ids, len(device_ids))
        else:
            rc = lib.axon_start_nrt_profile(None, 0)
        if rc != 0:
            raise RuntimeError(f"axon_start_nrt_profile rc={rc}")
        try:
            yield
        finally:
            n = lib.axon_stop_nrt_profile(str(output_dir).encode())
            print(f"ntff profile: {n} file(s) written to {output_dir}",
                  file=sys.stderr)

    mod = types.ModuleType("antenv.axon_hooks")
    _h = [_hook]
    mod.set_axon_ntff_profile_hook = lambda h: _h.__setitem__(0, h)
    mod.get_axon_ntff_profile_hook = lambda: _h[0]
    sys.modules["antenv.axon_hooks"] = mod
    import antenv
    antenv.axon_hooks = mod


def build():
    nc = bacc.Bacc("TRN2", target_bir_lowering=False, debug=False,
                   num_devices=N_CORES)

    xT_d = nc.dram_tensor("xT", [D, NT], BF16, kind="ExternalInput")
    wq_d = nc.dram_tensor("wqT", [D, FL], BF16, kind="ExternalInput")
    wk_d = nc.dram_tensor("wkT", [D, FL], BF16, kind="ExternalInput")
    wv_d = nc.dram_tensor("wvT", [D, FL], BF16, kind="ExternalInput")
    ow_d = nc.dram_tensor("owT", [FL, D], BF16, kind="ExternalInput")
    bq_d = nc.dram_tensor("bq", [128, FC], F32, kind="ExternalInput")
    bk_d = nc.dram_tensor("bk", [128, FC], F32, kind="ExternalInput")
    out_d = nc.dram_tensor("outT", [D, NT], F32, kind="ExternalOutput")

    chunked = lambda t: t.ap().rearrange("(c p) t -> p c t", p=128)

    with tile.TileContext(nc) as tc:
        with tc.tile_pool(name="persist", bufs=1) as persist:
            kT = persist.tile([128, FC, NT], BF16)
            qT = persist.tile([128, FC, NT], BF16)
            v = persist.tile([128, NKT, NH, HD + 1], BF16)
            attn = persist.tile([128, FC, NT], BF16)
            bq_sb = persist.tile([128, FC], F32)
            bk_sb = persist.tile([128, FC], F32)
            nc.vector.memset(v, 1.0)
            warm = persist.tile([128, 1], F32)
            nc.vector.memset(warm, 0.0)

            # PSUM budget (8 banks): ps_acc 2x[128,512] proj/out accumulators,
            # ps_s 2x[128,2,512] scores, ps_o 2x[65,512] PV accumulators.
            with tc.tile_pool(name="w1", bufs=1) as w1, \
                 tc.tile_pool(name="xpool", bufs=1) as xpool, \
                 tc.tile_pool(name="ppool", bufs=17) as ppool, \
                 tc.tile_pool(name="nrm", bufs=2) as nrm, \
                 tc.tile_pool(name="fout", bufs=3) as fout, \
                 tc.tile_pool(name="drpool", bufs=4, space="DRAM") as drpool, \
                 tc.tile_pool(name="ps_acc", bufs=2, space="PSUM") as ps_acc, \
                 tc.tile_pool(name="ps_s", bufs=2, space="PSUM") as ps_s, \
                 tc.tile_pool(name="ps_o", bufs=2, space="PSUM") as ps_o:
                xT = xpool.tile([128, DC, NT], BF16)
                wq = w1.tile([128, DC, FL], BF16, tag="wq")
                wk = w1.tile([128, DC, FL], BF16, tag="wk")
                wv = w1.tile([128, DC, FL], BF16, tag="wv")
                ow = w1.tile([128, FC, D], BF16, tag="ow")

                # DMA: mid-grain transfers spread over three queues — large
                # single DMAs serialize on one engine, while 40 tiny loads
                # gate on the ~600ns per-dma_start issue rate of one queue.
                # xT tc0 pieces land first so the first K chain starts ~6us
                # in; wk per-dc on the ACT queue in parallel.
                nc.scalar.dma_start(out=bq_sb, in_=bq_d.ap())
                nc.scalar.dma_start(out=bk_sb, in_=bk_d.ap())
                for dc in range(DC):
                    nc.scalar.dma_start(out=wk[:, dc, :],
                                        in_=chunked(wk_d)[:, dc, :])
                for tc_i in range(NQC):
                    tsl = slice(tc_i * QC, (tc_i + 1) * QC)
                    nc.sync.dma_start(out=xT[:, 0:4, tsl],
                                      in_=chunked(xT_d)[:, 0:4, tsl])
                    nc.gpsimd.dma_start(out=xT[:, 4:8, tsl],
                                        in_=chunked(xT_d)[:, 4:8, tsl])
                nc.scalar.dma_start(out=wq, in_=chunked(wq_d))
                nc.scalar.dma_start(out=wv, in_=chunked(wv_d))
                nc.scalar.dma_start(out=ow, in_=chunked(ow_d))
                # dummy exp pulls the ACT_TABLE_LOAD off the first real
                # score tile's critical path
                nc.scalar.activation(warm, warm, EXP)

                # ---- projection chains (8 matmuls + epilogue each) ----
                def k_chain(fc, tc_i):
                    tsl = slice(tc_i * QC, (tc_i + 1) * QC)
                    ps = ps_acc.tile([128, QC], F32, tag="ps")
                    for dc in range(DC):
                        yield nc.tensor.matmul(
                            ps, lhsT=wk[:, dc, fc * 128:(fc + 1) * 128],
                            rhs=xT[:, dc, tsl],
                            start=(dc == 0), stop=(dc == DC - 1))
                    yield nc.vector.tensor_scalar_add(
                        kT[:, fc, tsl], ps, bk_sb[:, fc:fc + 1])

                def q_chain(fc, tc_i):
                    tsl = slice(tc_i * QC, (tc_i + 1) * QC)
                    ps = ps_acc.tile([128, QC], F32, tag="ps")
                    for dc in range(DC):
                        yield nc.tensor.matmul(
                            ps, lhsT=wq[:, dc, fc * 128:(fc + 1) * 128],
                            rhs=xT[:, dc, tsl],
                            start=(dc == 0), stop=(dc == DC - 1))
                    yield nc.vector.tensor_scalar_add(
                        qT[:, fc, tsl], ps, bq_sb[:, fc:fc + 1])

                def v_chain(tt):
                    ps = ps_acc.tile([128, QC], F32, tag="ps")
                    for dc in range(DC):
                        yield nc.tensor.matmul(
                            ps, lhsT=xT[:, dc, tt * 128:(tt + 1) * 128],
                            rhs=wv[:, dc, :],
                            start=(dc == 0), stop=(dc == DC - 1))
                    yield nc.vector.tensor_copy(
                        out=v[:, tt, :, 0:HD],
                        in_=ps.rearrange("p (h d) -> p h d", d=HD))

                def out_chain(ec, tc_i):
                    tsl = slice(tc_i * QC, (tc_i + 1) * QC)
                    ps = ps_acc.tile([128, QC], F32, tag="ps")
                    for fc in range(FC):
                        yield nc.tensor.matmul(
                            ps, lhsT=ow[:, fc, ec * 128:(ec + 1) * 128],
                            rhs=attn[:, fc, tsl],
                            start=(fc == 0), stop=(fc == FC - 1))
                    fo = fout.tile([128, QC], F32, tag="fo")
                    yield nc.vector.tensor_copy(out=fo, in_=ps)
                    # gpsimd queue: keeps the big output transfers from
                    # delaying the normalization DMAs on the sync queue
                    yield nc.gpsimd.dma_start(
                        out=out_d.ap()[ec * 128:(ec + 1) * 128, tsl], in_=fo)

                # Deadline-ordered filler queue of (key, generator); attn
                # units pop a couple of steps per k-tile slot to keep the PE
                # dense while ACT owns the critical path.  Correctness rule:
                # everything a unit's own matmuls READ must be fully emitted
                # before the unit emits them (the PE executes in order, so a
                # score matmul parked on a not-yet-emitted chain's epilogue
                # deadlocks the queue) — require() force-drains those.
                filler = deque()
                done_keys = set()

                def push(key, gen):
                    filler.append((key, gen))

                def drain(n):
                    for _ in range(n):
                        if not filler:
                            return
                        key, gen = filler[0]
                        try:
                            next(gen)
                        except StopIteration:
                            done_keys.add(key)
                            filler.popleft()

                def drain_all():
                    while filler:
                        drain(1)

                def require(*keys):
                    while any(k not in done_keys for k in keys):
                        assert filler, f"missing filler chains: {keys}"
                        drain(1)

                def attn_unit(p, qc, first=False, fill=2, extra=()):
                    # Cascaded schedule: EVERY unit defers its 16 PV matmul
                    # pairs + normalization into the NEXT unit's slots (the
                    # `extra` thunks, flushed two per slot between the gated
                    # score matmuls). This keeps ready PE work between every
                    # exp-gated instruction and moves each unit's PSUM
                    # evacuation safely after its last PV in queue order.
                    # K chunks are required in kt-stages (kt//4 == tc) so the
                    # first scores don't wait on the whole 2048-token K.
                    require(("k", p, 0), ("q", p, qc))
                    if not first:
                        # this unit flushes the previous unit's PV thunks,
                        # which read v: every v chain must be emitted first
                        require(*[("v", tt) for tt in range(NKT)])
                    he, ho = 2 * p, 2 * p + 1
                    qsl = slice(qc * QC, (qc + 1) * QC)
                    po_e = ps_o.tile([HD + 1, QC], F32, tag="po")
                    po_o = ps_o.tile([HD + 1, QC], F32, tag="po")

                    def pv(pt, kt):
                        nc.tensor.matmul(
                            po_e, lhsT=v[:, kt, he, :], rhs=pt[:, 0, :],
                            start=(kt == 0), stop=(kt == NKT - 1))
                        nc.tensor.matmul(
                            po_o, lhsT=v[:, kt, ho, :], rhs=pt[:, 1, :],
                            start=(kt == 0), stop=(kt == NKT - 1))

                    extra = deque(extra)
                    backlog = []
                    for kt in range(NKT):
                        if kt % 4 == 0 and kt > 0:
                            require(("k", p, kt // 4))
                        ss = ps_s.tile([128, SB, QC], F32, tag="ss")
                        for j in range(SB):
                            hi = j * 64
                            nc.tensor.matmul(
                                ss[:, j, :],
                                lhsT=kT[hi:hi + HD, p,
                                        kt * 128:(kt + 1) * 128],
                                rhs=qT[hi:hi + HD, p, qsl],
                                start=True, stop=True)
                        pt = ppool.tile([128, SB, QC], BF16, tag="pt",
                                        bufs=19)
                        nc.scalar.activation(pt, ss, EXP, scale=0.125)
                        backlog.append((pt, kt))
                        for _ in range(2):
                            if extra:
                                extra.popleft()()
                        drain(fill)
                    while extra:
                        extra.popleft()()

                    def norm():
                        _norm(p, qc, po_e, po_o)

                    return ([lambda a=a, b=b: pv(a, b)
                             for a, b in backlog] + [norm])

                def _norm(p, qc, po_e, po_o):
                    # normalization: evacuate both PV accumulators, batch the
                    # two 1/sum rows into one reciprocal, DRAM-bounce the
                    # partition broadcast, multiply.
                    qsl = slice(qc * QC, (qc + 1) * QC)
                    ps_e = nrm.tile([HD + 1, QC], F32, tag="ps_sb", bufs=3)
                    nc.vector.tensor_copy(out=ps_e, in_=po_e)
                    ps_o_sb = nrm.tile([HD + 1, QC], F32, tag="ps_sb",
                                       bufs=3)
                    nc.vector.tensor_copy(out=ps_o_sb, in_=po_o)
                    # partition-gather the two sums rows via DMA (DVE ops
                    # cannot shift partition bases), one reciprocal for both
                    sr = nrm.tile([2, QC], F32, tag="sr")
                    nc.sync.dma_start(out=sr[0:1, :], in_=ps_e[HD:HD + 1, :])
                    nc.sync.dma_start(out=sr[1:2, :],
                                      in_=ps_o_sb[HD:HD + 1, :])
                    rc = nrm.tile([2, QC], F32, tag="rc")
                    nc.vector.reciprocal(rc, sr)
                    dr = drpool.tile([2, QC], F32, tag="dr")
                    nc.sync.dma_start(out=dr, in_=rc)
                    bc_e = nrm.tile([64, QC], F32, tag="bc_e")
                    nc.sync.dma_start(
                        out=bc_e,
                        in_=bass.AP(tensor=dr.tensor, offset=dr.offset,
                                    ap=[[0, 64], dr.ap[-1]]))
                    bc_o = nrm.tile([64, QC], F32, tag="bc_o")
                    nc.sync.dma_start(
                        out=bc_o,
                        in_=bass.AP(tensor=dr.tensor,
                                    offset=dr.offset + QC,
                                    ap=[[0, 64], dr.ap[-1]]))
                    nc.vector.tensor_tensor(
                        out=attn[0:HD, p, qsl],
                        in0=ps_e[0:HD, :], in1=bc_e, op=MULT)
                    sh = nrm.tile([64, QC], BF16, tag="sh")
                    nc.vector.tensor_tensor(
                        out=sh, in0=ps_o_sb[0:HD, :], in1=bc_o, op=MULT)
                    nc.sync.dma_start(out=attn[64:128, p, qsl], in_=sh)

                # ---- emission ----
                # preamble: only K(0, tc0) + Q(0, qc0) gate the first scores
                push(("k", 0, 0), k_chain(0, 0))
                push(("q", 0, 0), q_chain(0, 0))
                require(("k", 0, 0), ("q", 0, 0))

                # unit 1 runs with the remaining K chunks + the V projection
                # as its filler (PV deferred to its tail)
                for tc_i in range(1, NQC):
                    push(("k", 0, tc_i), k_chain(0, tc_i))
                for tt in range(NKT):
                    push(("v", tt), v_chain(tt))
                push(("q", 0, 1), q_chain(0, 1))
                for tc_i in range(NQC):
                    push(("k", 1, tc_i), k_chain(1, tc_i))
                push(("q", 1, 0), q_chain(1, 0))
                bl = attn_unit(0, 0, first=True, fill=9)

                push(("q", 1, 1), q_chain(1, 1))
                push(("q", 0, 2), q_chain(0, 2))
                bl = attn_unit(0, 1, extra=bl)
                push(("q", 0, 3), q_chain(0, 3))
                push(("q", 1, 2), q_chain(1, 2))
                bl = attn_unit(1, 0, extra=bl)
                for tc_i in range(NQC):
                    push(("k", 2, tc_i), k_chain(2, tc_i))
                bl = attn_unit(1, 1, extra=bl)
                push(("q", 1, 3), q_chain(1, 3))
                push(("q", 2, 0), q_chain(2, 0))
                bl = attn_unit(0, 2, extra=bl)
                push(("q", 2, 1), q_chain(2, 1))
                bl = attn_unit(0, 3, extra=bl)
                for tc_i in range(NQC):
                    push(("k", 3, tc_i), k_chain(3, tc_i))
                bl = attn_unit(1, 2, extra=bl)
                push(("q", 3, 0), q_chain(3, 0))
                push(("q", 3, 1), q_chain(3, 1))
                bl = attn_unit(1, 3, extra=bl)
                push(("q", 2, 2), q_chain(2, 2))
                push(("q", 2, 3), q_chain(2, 3))
                bl = attn_unit(2, 0, extra=bl)
                push(("q", 3, 2), q_chain(3, 2))
                push(("q", 3, 3), q_chain(3, 3))
                bl = attn_unit(2, 1, extra=bl)
                bl = attn_unit(3, 0, extra=bl)
                bl = attn_unit(3, 1, extra=bl)
                # qc0 attn for all pairs completes inside unit (3,1) (it
                # flushes (3,0)'s PV+norm) -> out-proj tc0 can follow
                for ec in range(DC):
                    push(("o", ec, 0), out_chain(ec, 0))
                bl = attn_unit(2, 2, extra=bl)
                for ec in range(DC):
                    push(("o", ec, 1), out_chain(ec, 1))
                bl = attn_unit(2, 3, extra=bl)
                bl = attn_unit(3, 2, extra=bl, fill=3)
                bl = attn_unit(3, 3, extra=bl, fill=2)
                for t in bl:          # last unit's PV + normalization
                    t()
                # tc2 out-chains only need (3,2)'s norm (flushed inside
                # (3,3)) — they execute during (3,3)'s normalization
                # latency, keeping the HAM clock gate warm so the tc3
                # tail runs at full clock
                for ec in range(DC):
                    push(("o", ec, 2), out_chain(ec, 2))
                drain_all()
                for ec in range(DC):
                    push(("o", ec, 3), out_chain(ec, 3))
                drain_all()

    nc.compile()
    return nc


def _prep_in_maps(x, qkv_w, qkv_b, out_w, out_b):
    bf = ml_dtypes.bfloat16
    xTs = [np.ascontiguousarray(x[b].T).astype(bf) for b in range(4)]
    wqT, wkT, wvT, owT, bq, bk = [], [], [], [], [], []
    for hh in range(2):
        fsl = slice(hh * FL, (hh + 1) * FL)
        wqT.append(np.ascontiguousarray(qkv_w[0:D][fsl].T).astype(bf))
        wkT.append(np.ascontiguousarray(qkv_w[D:2 * D][fsl].T).astype(bf))
        wvT.append(np.ascontiguousarray(qkv_w[2 * D:3 * D][fsl].T).astype(bf))
        owT.append(np.ascontiguousarray(out_w.T[fsl]).astype(bf))
        bq.append(np.ascontiguousarray(
            qkv_b[0:D][fsl].reshape(FC, 128).T).astype(np.float32))
        bk.append(np.ascontiguousarray(
            qkv_b[D:2 * D][fsl].reshape(FC, 128).T).astype(np.float32))

    in_maps = []
    for i in range(N_CORES):
        b, hh = i // 2, i % 2
        in_maps.append(dict(xT=xTs[b], wqT=wqT[hh], wkT=wkT[hh],
                            wvT=wvT[hh], owT=owT[hh], bq=bq[hh], bk=bk[hh]))
    return in_maps


def run(x, qkv_w, qkv_b, out_w, out_b, trace=False):
    if trace:
        _install_ntff_shim()
    if "nc" not in _CACHE:
        _CACHE["nc"] = build()
    nc = _CACHE["nc"]
    x = np.asarray(x, np.float32)
    qkv_w = np.asarray(qkv_w, np.float32)
    qkv_b = np.asarray(qkv_b, np.float32)
    out_w = np.asarray(out_w, np.float32)
    out_b = np.asarray(out_b, np.float32)
    in_maps = _prep_in_maps(x, qkv_w, qkv_b, out_w, out_b)
    res = run_bass_kernel_spmd(nc, in_maps, core_ids=list(range(N_CORES)),
                               trace=trace)
    # host: sum the two head-half partials per batch, add bv-folded bias
    ob_eff = (out_b + out_w @ qkv_b[2 * D:3 * D]).astype(np.float32)
    out = np.empty((4, NT, D), np.float32)
    for b in range(4):
        acc = res.results[2 * b]["outT"] + res.results[2 * b + 1]["outT"]
        out[b] = acc.T + ob_eff
    return out, res


def kernel(**inputs):
    out, _ = run(**inputs)
    return out
